# revision 63
# baseline (speedup 1.0000x reference)
"""Causal self-attention kernel for Trainium2 (8 NeuronCores, Bass/Tile).

Problem: B=4, S=2048, D=1024, H=16, HD=64, fp32.
Sharding: core c -> (batch b = c//2, head-group hg = c%2). Each core computes
attention for its batch over 8 heads (features hg*512..hg*512+511 of each of
the k/q/v projection chunks), plus the partial output projection
attn_out_slice @ W_out[rows of this head group].  Host sums the two partial
out-projections per batch and adds nothing else (b_out folded in on hg==0).

Device-side layout choices (no on-device transposes anywhere):
  - host provides x^T [D, S]; K^T/Q^T are produced feature-major [F, S] in
    bf16 by using W as the matmul stationary operand; V is produced
    seq-major [S, 8*65] by using x^T as the stationary operand, with a
    constant-1.0 65th column per head.
  - attention uses the scores-transposed layout S^T[k, q]: QK^T pairs of
    heads run row-tiled (head A in PE rows 0-63, head B in rows 64-127,
    concurrent on HW), exp() on the scalar engine (no max subtraction:
    scores ~ N(0,1)), causal masking as a 0/1 multiply on band tiles only.
  - AV matmuls use M=65 stationaries [v_head | 1]: the 65th output
    partition accumulates the softmax denominator for free (no separate
    ones-matmuls).  1/D is broadcast across the 64 feature rows via a tiny
    DRAM round-trip DMA (0-step partition APs are DRAM-source-only), and
    head B's normalized tile is shifted to partitions 64-127 by an
    SBUF->SBUF DMA (DVE cannot cross lanes).
  - software pipelining: AV trails QK/exp by one k-tile; each q-block's
    normalize/evict chain is emitted just after the next q-block's first
    k-tile (early DVE start), and all out-projection chunks run at the
    tail where the last chain's latency hides under them.
  - DMAs are batched into few multi-dim-AP transfers (a dma_start costs
    ~0.6us of issuing-engine sequencer time), ordered so the first V-proj
    matmuls start ~3us after launch.
"""

import math
from contextlib import ExitStack

import numpy as np
from ml_dtypes import bfloat16

import concourse.bass as bass
import concourse.tile as tile
from concourse import bacc, mybir
from concourse.bass_utils import run_bass_kernel_spmd

F32 = mybir.dt.float32
BF16 = mybir.dt.bfloat16

def build_nc(S=2048, D=1024, H_pc=8, HD=64, NQ=512, KT=128, reps=1):
    """Build the single-core Bass program (identical program on all cores).

    reps>1 wraps the whole kernel body in an on-device For_i loop; used only
    for slope-based HW timing (delta wall-time per rep through the axon
    tunnel), never for the graded path.
    """
    F = H_pc * HD          # per-core feature width of each of k/q/v (512)
    HP = F // 128          # head-pairs == 128-wide feature tiles (4)
    DKT = D // 128         # contraction tiles over d_model (8)
    NSEQ = S // NQ         # q blocks (4)
    NST = S // KT          # seq tiles for V (16)
    DM = D // 128          # output d_model tiles (8)
    NCH = S // 512         # 512-wide seq chunks for projections (4)
    BAND = NQ // KT        # k-tiles per q-block on the causal diagonal (4)

    nc = bacc.Bacc("TRN2", target_bir_lowering=False, debug=False, num_devices=8)

    x_t = nc.dram_tensor("x_t", [D, S], BF16, kind="ExternalInput").ap()
    w_k = nc.dram_tensor("w_k", [D, F], BF16, kind="ExternalInput").ap()
    w_q = nc.dram_tensor("w_q", [D, F], BF16, kind="ExternalInput").ap()
    w_v = nc.dram_tensor("w_v", [D, F], BF16, kind="ExternalInput").ap()
    b_k = nc.dram_tensor("b_k", [F, 1], F32, kind="ExternalInput").ap()
    b_q = nc.dram_tensor("b_q", [F, 1], F32, kind="ExternalInput").ap()
    b_v = nc.dram_tensor("b_v", [F], F32, kind="ExternalInput").ap()
    w_o = nc.dram_tensor("w_o", [F, D], BF16, kind="ExternalInput").ap()
    b_o = nc.dram_tensor("b_o", [D, 1], F32, kind="ExternalInput").ap()
    masks = nc.dram_tensor("masks", [128, 2, 128], BF16, kind="ExternalInput").ap()
    out_t = nc.dram_tensor("out_t", [D, S], F32, kind="ExternalOutput").ap()

    scale = 1.0 / math.sqrt(HD)

    with tile.TileContext(nc) as tc, ExitStack() as ctx:
        consts = ctx.enter_context(tc.tile_pool(name="consts", bufs=1))
        # per-partition bias columns for the feature-major K/Q projections
        bk_sb = consts.tile([128, HP], F32, tag="bk")
        bq_sb = consts.tile([128, HP], F32, tag="bq")
        nc.sync.dma_start(out=bk_sb, in_=b_k.rearrange("(m p) one -> p (m one)", p=128))
        nc.sync.dma_start(out=bq_sb, in_=b_q.rearrange("(m p) one -> p (m one)", p=128))
        # V bias broadcast along partitions (bias varies along the free dim)
        bv_sb = consts.tile([128, F], F32, tag="bv")
        bv_bcast = bass.AP(tensor=b_v.tensor, offset=b_v.offset, ap=[[0, 128], [1, F]])
        nc.sync.dma_start(out=bv_sb, in_=bv_bcast)
        bo_sb = consts.tile([128, DM], F32, tag="bo")
        nc.sync.dma_start(out=bo_sb, in_=b_o.rearrange("(m p) one -> p (m one)", p=128))


        # persistent activations.  v stores 65 columns per head: 64 features
        # plus a constant-1 column, so the AV matmul's 65th output partition
        # accumulates the softmax denominator for free.
        HD1 = HD + 1
        FV = H_pc * HD1        # 520
        big = ctx.enter_context(tc.tile_pool(name="big", bufs=1))
        kT = [big.tile([128, S], BF16, tag=f"kT{m}", name=f"kT{m}") for m in range(HP)]
        qT = [big.tile([128, S], BF16, tag=f"qT{m}", name=f"qT{m}") for m in range(HP)]
        v = [big.tile([128, FV], BF16, tag=f"v{st}", name=f"v{st}") for st in range(NST)]
        aT = [big.tile([128, S], BF16, tag=f"aT{m}", name=f"aT{m}") for m in range(HP)]
        # fill v tiles with 1.0 (idle-DVE memsets at startup); the
        # V-projection eviction overwrites the 64 feature columns per head,
        # leaving each head's 65th (denominator) column at 1.0.
        for st in range(NST):
            nc.vector.memset(v[st][:, :], 1.0)

        # ---- Phases A+B interleaved: V, then per head-pair {K,Q proj; attention} ----
        # All [128,512] PSUM accumulations (V-proj, K/Q-proj, scores) share one
        # 4-buffer pool so projection and attention pipelines coexist in the
        # 8 PSUM banks and the scheduler can overlap them across head-pairs.
        xp = ctx.enter_context(tc.tile_pool(name="xp", bufs=1))
        mk = ctx.enter_context(tc.tile_pool(name="mk", bufs=1))
        wsp = ctx.enter_context(tc.tile_pool(name="wsp", bufs=2 * 4))
        ptp = ctx.enter_context(tc.tile_pool(name="pt_pool", bufs=5))
        r2p = ctx.enter_context(tc.tile_pool(name="r2_pool", bufs=2))
        sp = ctx.enter_context(tc.tile_pool(name="sp", bufs=2, space="PSUM"))
        op = ctx.enter_context(tc.tile_pool(name="op", bufs=2, space="PSUM"))
        wop = ctx.enter_context(tc.tile_pool(name="wop", bufs=1))
        osb = ctx.enter_context(tc.tile_pool(name="os", bufs=2))
        wvp = ctx.enter_context(tc.tile_pool(name="wvp", bufs=1))
        rdp = ctx.enter_context(tc.tile_pool(name="rdp", bufs=2, space="DRAM"))

        def body():
            # one [128, DKT, S] tile for x^T; dma_start issue overhead is
            # ~0.6us of engine-sequencer time each, so batch all loads into
            # few multi-dim-AP DMAs (DRAM side: p stride S, k stride 128*S).
            xt8 = xp.tile([128, DKT * S], BF16, tag="x8", name="x8")
            xt = [xt8[:, k * S:(k + 1) * S] for k in range(DKT)]

            wv8 = wvp.tile([128, DKT * F], BF16, tag="wv8", name="wv8")
            wv = [wv8[:, k * F:(k + 1) * F] for k in range(DKT)]
            # startup-latency-ordered loads: a small first x slice, then the
            # V weights in halves, then the rest of x — so the first V-proj
            # matmuls can start ~3us in instead of ~10us
            x3 = xt8.rearrange("p (k s) -> p k s", k=DKT)

            def x_load(c0, c1):
                nc.scalar.dma_start(out=x3[:, :, c0:c1], in_=bass.AP(
                    tensor=x_t.tensor, offset=x_t.offset + c0,
                    ap=[[S, 128], [128 * S, DKT], [1, c1 - c0]]))

            x_load(0, 256)
            for h in range(2):
                nc.scalar.dma_start(
                    out=wv8[:, h * 4 * F:(h + 1) * 4 * F], in_=bass.AP(
                        tensor=w_v.tensor, offset=w_v.offset + h * 4 * 128 * F,
                        ap=[[F, 128], [128 * F, 4], [1, F]]))
            x_load(256, 512)
            for c in range(1, NCH):
                x_load(c * 512, (c + 1) * 512)
            maskt = mk.tile([128, 2, 128], BF16, tag="mask", name="maskt")
            nc.scalar.dma_start(out=maskt, in_=masks)
            for st in range(NST):
                ps = sp.tile([128, 2 * NQ], F32, tag="s")
                for k in range(DKT):
                    nc.tensor.matmul(
                        ps[:, 0:F], xt[k][:, st * 128:(st + 1) * 128], wv[k],
                        start=(k == 0), stop=(k == DKT - 1),
                    )
                v3 = v[st].rearrange("p (h c) -> p h c", c=HD1)
                ps3 = ps[:, 0:F].rearrange("p (h c) -> p h c", c=HD)
                bv3 = bv_sb.rearrange("p (h c) -> p h c", c=HD)
                nc.vector.tensor_add(v3[:, :, 0:HD], ps3, bv3)

            # preload all K/Q projection weights (16 KB SBUF) so no head-pair
            # transition ever waits on a weight DMA; emitted after the V
            # phase so these transfers queue behind the startup x chunks
            wkq = {}
            for php in range(HP):
                for wi, wdram in ((0, w_k), (1, w_q)):
                    wt8 = wsp.tile([128, DKT * 128], BF16, tag="w", name="wt")
                    nc.sync.dma_start(out=wt8, in_=bass.AP(
                        tensor=wdram.tensor, offset=wdram.offset + php * 128,
                        ap=[[F, 128], [128 * F, DKT], [1, 128]]))
                    wkq[(php, wi)] = [wt8[:, k * 128:(k + 1) * 128]
                                      for k in range(DKT)]

            cA = slice(0, 64)
            cB = slice(64, 128)

            # Output projection per 512-chunk through the shared score pool;
            # W_out for this core is 1 MB bf16: preload it fully.
            wot = [wop.tile([128, D], BF16, tag=f"wot{k}", name=f"wot{k}")
                   for k in range(HP)]
            wo = [[wot[k][:, mo * 128:(mo + 1) * 128] for mo in range(DM)]
                  for k in range(HP)]
            for k in range(HP):
                nc.sync.dma_start(out=wot[k], in_=w_o[k * 128:(k + 1) * 128, :])

            def outproj_chunk(nch):
                # 4 mo-blocks share one SBUF tile and one (3D-AP) store DMA
                for g in range(DM // 4):
                    og = osb.tile([128, 4 * 512], F32, tag="ot")
                    for i in range(4):
                        mo = g * 4 + i
                        ps = sp.tile([128, 2 * NQ], F32, tag="s")
                        for k in range(HP):
                            nc.tensor.matmul(
                                ps[:, 0:512], wo[k][mo],
                                aT[k][:, nch * 512:(nch + 1) * 512],
                                start=(k == 0), stop=(k == HP - 1),
                            )
                        nc.vector.tensor_scalar_add(
                            og[:, i * 512:(i + 1) * 512], ps[:, 0:512],
                            bo_sb[:, mo:mo + 1])
                    nc.sync.dma_start(out=bass.AP(
                        tensor=out_t.tensor,
                        offset=out_t.offset + g * 4 * 128 * S + nch * 512,
                        ap=[[S, 128], [128 * S, 4], [1, 512]]), in_=og)

            pending_dve = []   # deferred normalize/evict chains

            for hp in range(HP):
                # K and Q projections for this head-pair's feature tile
                for nch in range(NCH):
                    # chunk-major, K/Q interleaved: attention's first QK needs
                    # only (K chunk 0, Q chunk 0), so it starts ~2.5x earlier
                    for (wi, bias_sb, dstT) in ((0, bk_sb, kT), (1, bq_sb, qT)):
                        wt = wkq[(hp, wi)]
                        ps = sp.tile([128, 2 * NQ], F32, tag="s")
                        for k in range(DKT):
                            nc.tensor.matmul(
                                ps[:, 0:512], wt[k], xt[k][:, nch * 512:(nch + 1) * 512],
                                start=(k == 0), stop=(k == DKT - 1),
                            )
                        with nc.allow_low_precision(reason="bf16 k/q"):
                            nc.vector.tensor_scalar_add(
                                dstT[hp][:, nch * 512:(nch + 1) * 512],
                                ps[:, 0:512], bias_sb[:, hp:hp + 1],
                            )

                # attention for this head-pair
                for qi in range(NSEQ):
                    nkt = (qi + 1) * BAND
                    # o holds head A in bank 0 cols, head B in bank 1 cols;
                    # partition 64 of each accumulates the softmax denominator
                    # (the constant-1 column of v).
                    o = op.tile([128, 2 * NQ], F32, tag="o")
                    qs = slice(qi * NQ, (qi + 1) * NQ)

                    def av_pair(kt, lo, pt):
                        first, last = (kt == 0), (kt == nkt - 1)
                        nc.tensor.matmul(
                            o[0:65, lo:NQ],
                            v[kt][:, hp * 2 * HD1:hp * 2 * HD1 + HD1],
                            pt[:, lo:NQ],
                            start=first, stop=last, tile_position=(0, 0),
                            skip_group_check=True,
                        )
                        nc.tensor.matmul(
                            o[0:65, NQ + lo:2 * NQ],
                            v[kt][:, hp * 2 * HD1 + HD1:hp * 2 * HD1 + 2 * HD1],
                            pt[:, NQ + lo:2 * NQ],
                            start=first, stop=last, tile_position=(0, 0),
                            skip_group_check=True,
                        )

                    prev_av = None  # AV runs one k-tile behind QK/exp so the
                    # in-order PE always has a QK ready while ACT does exp
                    for kt in range(nkt):
                        if kt == 1 and pending_dve:
                            # previous q-block's normalize/evict chain: emit
                            # early so the in-order DVE starts it immediately
                            # and it completes during this k-loop
                            pending_dve.pop(0)()
                        ks = slice(kt * 128, (kt + 1) * 128)
                        j = kt - (nkt - BAND)
                        # valid q-subrange of this k-tile: q_local >= 128*j
                        lo = 128 * j if j > 0 else 0
                        s2 = sp.tile([128, 2 * NQ], F32, tag="s")
                        qk_lo = lo
                        qsub = slice(qi * NQ + qk_lo, (qi + 1) * NQ)
                        nc.tensor.matmul(
                            s2[:, qk_lo:NQ], kT[hp][cA, ks], qT[hp][cA, qsub],
                            start=True, stop=True, tile_position=(0, 0),
                        )
                        nc.tensor.matmul(
                            s2[:, NQ + qk_lo:2 * NQ], kT[hp][cB, ks],
                            qT[hp][cB, qsub],
                            start=True, stop=True, tile_position=(64, 0),
                        )
                        pt = ptp.tile([128, 2 * NQ], BF16, tag="p")
                        s2_3 = s2.rearrange("p (h q) -> p h q", h=2)
                        pt_3 = pt.rearrange("p (h q) -> p h q", h=2)
                        nc.scalar.activation(
                            pt_3[:, :, lo:NQ], s2_3[:, :, lo:NQ],
                            mybir.ActivationFunctionType.Exp, scale=scale,
                        )
                        if j >= 0:
                            # triangle mask on the first 128 valid columns
                            nc.vector.tensor_mul(
                                pt_3[:, :, lo:lo + 128], pt_3[:, :, lo:lo + 128],
                                maskt,
                            )
                        if prev_av is not None:
                            av_pair(*prev_av)
                        prev_av = (kt, lo, pt)
                    av_pair(*prev_av)
                    # 1/denominator (row 64 of each half) -> DRAM-round-trip
                    # broadcast over the 64 feature rows -> normalize+evict.
                    # Split lag-1 software pipeline: the DVE/DMA chain is
                    # emitted just after the next q-block's first k-tile (so
                    # the in-order DVE starts it early); the out-projection it
                    # gates is emitted after that k-loop ends (so the in-order
                    # PE never head-of-line blocks on it).
                    def evict(hp=hp, qi=qi, o=o, qs=qs):
                        r2v = r2p.tile([65, 2 * NQ], F32, tag="r2v")
                        nc.vector.reciprocal(r2v[64:65, :], o[64:65, 0:2 * NQ])
                        # broadcast 1/D across the 64 feature rows via a DRAM
                        # round-trip (0-step partition APs are DRAM-only);
                        # costs two small DMAs on otherwise-idle queues and no
                        # PE/PSUM at all.
                        rd = rdp.tile([1, 2 * NQ], F32, tag="rd")
                        nc.sync.dma_start(out=rd, in_=r2v[64:65, :])
                        rb = r2p.tile([64, 2 * NQ], F32, tag="rb")
                        rsrc = rd[:, :]
                        nc.sync.dma_start(out=rb, in_=bass.AP(
                            tensor=rsrc.tensor, offset=rsrc.offset,
                            ap=[[0, 64], [1, 2 * NQ]]))
                        nc.vector.tensor_mul(
                            aT[hp][cA, qs], o[0:64, 0:NQ], rb[:, 0:NQ])
                        stgB = r2p.tile([64, NQ], BF16, tag="stgB")
                        nc.vector.tensor_mul(
                            stgB, o[0:64, NQ:2 * NQ], rb[:, NQ:2 * NQ])
                        # partition shift 0-63 -> 64-127 (DVE can't cross lanes)
                        nc.sync.dma_start(out=aT[hp][cB, qs], in_=stgB)
                    pending_dve.append(evict)

            # tail: the last eviction chain's latency hides under the first
            # out-projection chunks (their aT inputs are long since ready)
            while pending_dve:
                pending_dve.pop(0)()
            for nch in range(NCH):
                outproj_chunk(nch)

        if reps == 1:
            body()
        else:
            with tc.For_i(0, reps, 1):
                body()

    nc.compile()
    return nc


def make_masks(NQ=512, KT=128):
    # triangle mask for the 128-wide causal boundary, duplicated for 2 heads
    k = np.arange(128)[:, None]
    c = np.arange(128)[None, :]
    keep = (c >= k).astype(np.float32)
    return np.stack([keep, keep], axis=1)  # [128, 2, 128]


def make_in_maps(x, W_in, b_in, W_out, b_out, S, D, H_pc, HD):
    """Build the 8 per-core input maps. Core c -> (batch c//2, head-group c%2)."""
    F = H_pc * HD
    B = x.shape[0]
    n_hg = D // F  # 2
    masks = make_masks()
    in_maps = []
    for c in range(B * n_hg):
        b, hg = c // n_hg, c % n_hg
        cols = slice(hg * F, (hg + 1) * F)
        # W_in chunk order (torch.chunk in the reference): k, q, v
        wk = np.ascontiguousarray(W_in[:, 0 * D:1 * D][:, cols])
        wq = np.ascontiguousarray(W_in[:, 1 * D:2 * D][:, cols])
        wv = np.ascontiguousarray(W_in[:, 2 * D:3 * D][:, cols])
        bk = np.ascontiguousarray(b_in[0 * D:1 * D][cols]).reshape(F, 1)
        bq = np.ascontiguousarray(b_in[1 * D:2 * D][cols]).reshape(F, 1)
        bv = np.ascontiguousarray(b_in[2 * D:3 * D][cols])
        wo = np.ascontiguousarray(W_out[cols, :])
        bo = (b_out if hg == 0 else np.zeros_like(b_out)).reshape(D, 1)
        in_maps.append({
            "x_t": np.ascontiguousarray(x[b].T).astype(bfloat16),
            "w_k": wk.astype(bfloat16), "w_q": wq.astype(bfloat16),
            "w_v": wv.astype(bfloat16),
            "b_k": bk.astype(np.float32), "b_q": bq.astype(np.float32),
            "b_v": bv.astype(np.float32),
            "w_o": wo.astype(bfloat16), "b_o": bo.astype(np.float32),
            "masks": masks.astype(bfloat16),
        })
    return in_maps


_NC_CACHE = {}


def _get_nc(key, **kw):
    if key not in _NC_CACHE:
        _NC_CACHE[key] = build_nc(**kw)
    return _NC_CACHE[key]


def kernel(x, W_in, b_in, W_out, b_out):
    x = np.asarray(x, dtype=np.float32)
    W_in = np.asarray(W_in, dtype=np.float32)
    b_in = np.asarray(b_in, dtype=np.float32)
    W_out = np.asarray(W_out, dtype=np.float32)
    b_out = np.asarray(b_out, dtype=np.float32)

    B, S, D = x.shape          # 4, 2048, 1024
    HD = 64
    H_pc = (D // HD) // 2      # 8 heads per core

    nc = _get_nc((S, D, H_pc), S=S, D=D, H_pc=H_pc, HD=HD)
    in_maps = make_in_maps(x, W_in, b_in, W_out, b_out, S, D, H_pc, HD)
    res = run_bass_kernel_spmd(nc, in_maps, list(range(2 * B)))
    outs = res.results
    out = np.empty((B, S, D), dtype=np.float32)
    for b in range(B):
        out[b] = (outs[2 * b]["out_t"] + outs[2 * b + 1]["out_t"]).T
    return out


def _pjrt_runner(nc, n_cores):
    """Cached jitted 8-core runner with no donation, for steady-state timing."""
    import jax
    from jax.sharding import Mesh, PartitionSpec, NamedSharding
    from jax.experimental.shard_map import shard_map
    from concourse import bass2jax, mybir as mb
    bass2jax.install_neuronx_cc_hook()

    partition_name = nc.partition_id_tensor.name if nc.partition_id_tensor else None
    in_names, out_names, out_avals, zero_outs = [], [], [], []
    for alloc in nc.m.functions[0].allocations:
        if not isinstance(alloc, mb.MemoryLocationSet):
            continue
        name = alloc.memorylocations[0].name
        if alloc.kind == "ExternalInput":
            if name != partition_name:
                in_names.append(name)
        elif alloc.kind == "ExternalOutput":
            out_names.append(name)
            shape = tuple(alloc.tensor_shape)
            dtype = mb.dt.np(alloc.dtype)
            out_avals.append(jax.core.ShapedArray(shape, dtype))
            zero_outs.append(np.zeros(shape, dtype))
    n_params = len(in_names)
    all_names = in_names + out_names
    if partition_name is not None:
        all_names = all_names + [partition_name]

    def _body(*args):
        operands = list(args)
        if partition_name is not None:
            operands.append(bass2jax.partition_id_tensor())
        outs = bass2jax._bass_exec_p.bind(
            *operands,
            out_avals=tuple(out_avals),
            in_names=tuple(all_names),
            out_names=tuple(out_names),
            lowering_input_output_aliases=(),
            sim_require_finite=True,
            sim_require_nnan=True,
            nc=nc,
        )
        return tuple(outs)

    devices = jax.devices()[:n_cores]
    mesh = Mesh(np.asarray(devices), ("core",))
    spec = PartitionSpec("core")
    f = jax.jit(shard_map(
        _body, mesh=mesh,
        in_specs=(spec,) * (n_params + len(out_names)),
        out_specs=(spec,) * len(out_names),
        check_rep=False,
    ))
    sharding = NamedSharding(mesh, spec)
    return f, in_names, zero_outs, sharding, out_names


def _timed_runner(reps, in_maps):
    """Jitted 8-core runner for the program with an on-device repeat loop."""
    import time as _time
    import jax
    nc = build_nc(reps=reps)
    f, in_names, zero_outs, sharding, out_names = _pjrt_runner(nc, len(in_maps))
    args = []
    for name in in_names:
        g = np.concatenate([np.asarray(in_maps[c][name]) for c in range(len(in_maps))], axis=0)
        args.append(jax.device_put(g, sharding))
    for z in zero_outs:
        g = np.concatenate([z] * len(in_maps), axis=0)
        args.append(jax.device_put(g, sharding))

    def run():
        t0 = _time.perf_counter()
        out = f(*args)
        jax.block_until_ready(out)
        return _time.perf_counter() - t0

    return run


def time_kernel(x, W_in, b_in, W_out, b_out, pairs=8, k2=65):
    """Measure (dispatch_wall_ns, hw_exec_ns).

    A single dispatch through the axon tunnel has a ~67 ms wall-clock floor
    of pure client-server latency (a trivial kernel measures the same), so
    the HW execution time is obtained as the slope of wall time vs on-device
    repeat count: (T(reps=k2) - T(reps=1)) / (k2 - 1), with the reps=1 calls
    interleaved around each reps=k2 call to cancel tunnel drift.
    """
    x = np.asarray(x, dtype=np.float32)
    B, S, D = x.shape
    HD = 64
    H_pc = (D // HD) // 2
    in_maps = make_in_maps(np.asarray(x), np.asarray(W_in), np.asarray(b_in),
                           np.asarray(W_out), np.asarray(b_out), S, D, H_pc, HD)
    r1 = _timed_runner(1, in_maps)
    r2 = _timed_runner(k2, in_maps)
    r1()
    r2()  # warmup
    slopes, walls = [], []
    for _ in range(pairs):
        t1a = r1()
        t2 = r2()
        t1b = r1()
        slopes.append((t2 - (t1a + t1b) / 2) / (k2 - 1) * 1e9)
        walls += [t1a, t1b]
    slopes.sort()
    return min(walls) * 1e9, slopes[len(slopes) // 2]

